# revision 28
# baseline (speedup 1.0000x reference)
"""BERT self-attention (B=8, S=1024, D=1024, H=16, Dh=64) on 8 NeuronCores.

Sharding: pure data parallel — core b handles batch element b (B == n_cores),
qkv_weight replicated. No collectives.

Per-core dataflow (all matmuls bf16 with fp32 PSUM accumulation):
  1. X [S,D] loaded first (prefetched 4 deep), cast to bf16 (DVE),
     PE-transposed into X^T [D,S] in groups of 4 chunks per PSUM unload;
     unloads alternate between DVE and ACT (idle early).
  2. W_v loaded+cast up front as [128, kt, 1024]; V computed into 2-bank
     [128,1024] PSUM tiles with stationary X^T chunks (128 matmuls), laid
     out as V' [S, H*(Dh+1)] where each head's 65th column carries
     exp(mask): softmax(s + m) == exp(s)*exp(m) normalized, so the additive
     mask is an exact per-key row scaling of V', and the extra column makes
     the PV matmul emit softmax denominators for free.
  3. Per head pair: W_q/W_k column slices loaded one pair ahead, Q^T,K^T
     computed as [features, S] into 2-bank PSUM tiles (one DVE unload each).
  4. Per head: scores^T [S_k,S_q] = (K^T chunk).T @ Q^T, two 512 chunks per
     2-bank PSUM tile; ACT computes exp(0.125*s) PSUM->SBUF(bf16) in single
     [128,1024] instructions;  ctx'^T [65,S_q] = V'.T @ expS^T; copied to
     SBUF bf16, PE-transposed (bf16) back to [S_q,65] four chunks per PSUM
     tile, one strided reciprocal per 4 denominators, cols 0..63 scaled by
     1/col64 on DVE (tensor_scalar_mul), keeping ACT exp-only.
  5. ctx assembled [S, D] fp32, DMA'd out in column groups as head quartets
     complete, hiding the output transfer behind compute.

DMA triggers are spread over both HWDGE rings (SP for bulk loads, ACT for
masks and output). PE emission order pipelines stages so the tensor engine
never waits on ACT/DVE results: score chunk-halves interleave with the
previous head's PV halves (so exp drain time is covered by PSUM-C work),
each head's ctx transposes ride one stage behind its PV, and PV(h) runs
after the next pair's QKV projection.

No max-subtraction in softmax: scores*scale is bounded (|x| <~ 4 for this
problem's scale) and exp runs in fp32 on ACT.
"""

import sys

import numpy as np

_REPO = "/opt/trn_rl_repo"
if _REPO not in sys.path:
    sys.path.insert(0, _REPO)

B, S, D, H, DH = 8, 1024, 1024, 16, 64
P = 128
NS = S // P          # seq tiles
NK = D // P          # contraction tiles
NHP = H // 2         # head pairs
NQ = 2               # 512-wide S_q chunks
QC = S // NQ         # 512
SCALE = 1.0 / 8.0    # 1/sqrt(DH)
VW = DH + 1          # V' live width per head (extra denominator column)
VP = DH + 2          # V' stored stride per head (pad for 4B-aligned slices)

_NC_CACHE = {}


def _build_nc():
    import concourse.bass as bass
    import concourse.tile as tile
    from concourse import bacc, mybir
    from concourse.masks import make_identity
    from contextlib import ExitStack

    f32 = mybir.dt.float32
    bf16 = mybir.dt.bfloat16
    Exp = mybir.ActivationFunctionType.Exp

    nc = bacc.Bacc("TRN2", target_bir_lowering=False, debug=False)
    x_d = nc.declare_dram_parameter("x", [S, D], f32, isOutput=False)
    w_d = nc.declare_dram_parameter("w", [D, 3 * D], f32, isOutput=False)
    m_d = nc.declare_dram_parameter("m", [S], f32, isOutput=False)
    o_d = nc.declare_dram_parameter("o", [S, D], f32, isOutput=True)

    with tile.TileContext(nc) as tc, ExitStack() as es:
        const = es.enter_context(tc.tile_pool(name="const", bufs=1))
        maskp = es.enter_context(tc.tile_pool(name="maskp", bufs=NS))
        xtp = es.enter_context(tc.tile_pool(name="xtp", bufs=1))
        vp = es.enter_context(tc.tile_pool(name="vp", bufs=NS))
        ctxp = es.enter_context(tc.tile_pool(name="ctxp", bufs=1))
        xstage = es.enter_context(tc.tile_pool(name="xstage", bufs=2))
        wvstage = es.enter_context(tc.tile_pool(name="wvstage", bufs=2))
        wvp = es.enter_context(tc.tile_pool(name="wvp", bufs=1))
        wstage = es.enter_context(tc.tile_pool(name="wstage", bufs=4))
        wqkp = es.enter_context(tc.tile_pool(name="wqkp", bufs=4))
        qktp = es.enter_context(tc.tile_pool(name="qktp", bufs=2))
        esp = es.enter_context(tc.tile_pool(name="esp", bufs=2 * NK))
        ctp = es.enter_context(tc.tile_pool(name="ctp", bufs=4))
        smallp = es.enter_context(tc.tile_pool(name="smallp", bufs=8))
        psA = es.enter_context(tc.tile_pool(name="psA", bufs=2, space="PSUM"))
        psB = es.enter_context(tc.tile_pool(name="psB", bufs=2, space="PSUM"))
        psC = es.enter_context(tc.tile_pool(name="psC", bufs=2, space="PSUM"))

        id_bf = const.tile([P, P], bf16, name="id_bf")
        make_identity(nc, id_bf)
        ones16 = const.tile([P, H], bf16, name="ones16")
        nc.vector.memset(ones16, 1.0)

        # persistent tensors
        xt = xtp.tile([P, NK, S], bf16, name="xt")  # X^T: [d-part, kt, s]
        v_sb = [vp.tile([P, H * VP], bf16, name=f"v{st}", tag="v") for st in range(NS)]
        ctx_all = ctxp.tile([P, NS, D], f32, name="ctx_all")
        ctx_sb = [ctx_all[:, st, :] for st in range(NS)]

        # X loads first on the SP ring (startup critical path); pair-0 W
        # slices after the first four tiles so QK0 can interleave with X^T
        xfs = []
        for i in range(NS):
            xf = xstage.tile([P, D], f32, name=f"xf{i}", tag="xf", bufs=4)
            nc.sync.dma_start(
                out=xf[:, 0:QC], in_=x_d[i * P:(i + 1) * P, 0:QC]
            )
            nc.sync.dma_start(
                out=xf[:, QC:D], in_=x_d[i * P:(i + 1) * P, QC:D]
            )
            xfs.append(xf)

        # X cast + PE transpose; PSUM unloads in groups of 4 chunks,
        # alternating DVE / ACT
        def emit_xt(i):
            xb = xstage.tile([P, D], bf16, name=f"xb{i}", tag="xb", bufs=2)
            for g in range(2):
                nc.vector.tensor_copy(
                    xb[:, g * QC:(g + 1) * QC], xfs[i][:, g * QC:(g + 1) * QC]
                )
                pst = psB.tile([P, 4 * P], bf16, name=f"px{i}_{g}", tag="psB")
                for c in range(4):
                    j = 4 * g + c
                    nc.tensor.transpose(
                        pst[:, c * P:(c + 1) * P], xb[:, j * P:(j + 1) * P], id_bf
                    )
                eng = nc.vector if (2 * i + g) % 2 else nc.scalar
                dst = xt[:, 4 * g:4 * g + 4, i * P:(i + 1) * P]
                srcp = pst.rearrange("p (c q) -> p c q", c=4)
                if eng is nc.vector:
                    eng.tensor_copy(dst, srcp)
                else:
                    eng.copy(dst, srcp)

        def qk_load(hp):
            # W_q/W_k column slices for this head pair: DMA + bf16 cast (DVE)
            wbf = []
            for t, base in enumerate((hp * P, D + hp * P)):
                wf = wstage.tile([P, NK, P], f32, name=f"wf{hp}_{t}", tag="wf")
                nc.sync.dma_start(
                    out=wf,
                    in_=w_d[:, base:base + P].rearrange("(kt p) c -> p kt c", p=P),
                )
                wb = wqkp.tile([P, NK, P], bf16, name=f"wb{hp}_{t}", tag="wb")
                nc.vector.tensor_copy(wb, wf)
                wbf.append(wb)
            return wbf

        # X^T for the first four tiles, then pair-0/1 W on the SP ring
        for i in range(4):
            emit_xt(i)
        wbf0 = qk_load(0)
        wfs1 = []
        for t, base in enumerate((P, D + P)):
            wf = wstage.tile([P, NK, P], f32, name=f"wf1_{t}", tag="wf")
            nc.sync.dma_start(
                out=wf,
                in_=w_d[:, base:base + P].rearrange("(kt p) c -> p kt c", p=P),
            )
            wfs1.append(wf)

        # W_v full load: [128, kt, 1024]
        wvb = wvp.tile([P, NK, D], bf16, name="wvb")
        wvfs = []
        for q in range(4):
            wvf = wvstage.tile([P, 2, D], f32, name=f"wvf{q}", tag="wvf")
            nc.sync.dma_start(
                out=wvf,
                in_=w_d[2 * q * P:(2 * q + 2) * P, 2 * D:3 * D].rearrange(
                    "(kt p) c -> p kt c", p=P
                ),
            )
            wvfs.append(wvf)

        # masks on the ACT ring; exp(mask) per seq tile, [128,1] scalars
        em = []
        for st in range(NS):
            msk = maskp.tile([P, 1], f32, name=f"msk{st}", tag="msk")
            nc.scalar.dma_start(
                out=msk,
                in_=m_d[st * P:(st + 1) * P].rearrange("(p o) -> p o", o=1),
            )
            emt = maskp.tile([P, 1], f32, name=f"em{st}", tag="em")
            nc.scalar.activation(emt, msk, Exp)
            em.append(emt)

        # V' denominator columns = exp(mask) per key row
        for st in range(NS):
            vcols = v_sb[st].rearrange("p (h c) -> p h c", h=H)[:, :, DH]
            nc.scalar.mul(vcols, ones16, em[st])

        def qk_chunks(hp, wbf):
            # QK as 8 four-matmul chunks into 1-bank psB tiles; each
            # (wsel, n) group is two chunks + a DVE unload, interleavable
            # between score tiles of the previous pair
            qt_t = qktp.tile([P, S], bf16, name=f"qt{hp}", tag="qt")
            kt_t = qktp.tile([P, S], bf16, name=f"kt{hp}", tag="kt")
            chunks = []
            for wsel, dest in ((1, kt_t), (0, qt_t)):
                for n in range(NQ):
                    cell = {}

                    def c0(cell=cell, wsel=wsel, n=n):
                        ps = psB.tile(
                            [P, QC], f32, name=f"pq{hp}_{wsel}_{n}", tag="psB"
                        )
                        cell["ps"] = ps
                        for k in range(4):
                            nc.tensor.matmul(
                                ps,
                                wbf[wsel][:, k, :],
                                xt[:, k, n * QC:(n + 1) * QC],
                                start=(k == 0),
                                stop=False,
                            )

                    def c1(cell=cell, wsel=wsel, n=n, dest=dest):
                        ps = cell["ps"]
                        for k in range(4, NK):
                            nc.tensor.matmul(
                                ps,
                                wbf[wsel][:, k, :],
                                xt[:, k, n * QC:(n + 1) * QC],
                                start=False,
                                stop=(k == NK - 1),
                            )
                        nc.vector.tensor_copy(
                            dest[:, n * QC:(n + 1) * QC], ps
                        )

                    chunks.append(c0)
                    chunks.append(c1)
            return qt_t, kt_t, chunks

        def emit_v_st(st):
            # V' [S, H*(Dh+2) padded]: stationary X^T chunks, 512-wide W_v
            for half in range(2):
                ps = psB.tile([P, QC], f32, name=f"pv{st}_{half}", tag="psB")
                for k in range(NK):
                    nc.tensor.matmul(
                        ps,
                        xt[:, k, st * P:(st + 1) * P],
                        wvb[:, k, half * QC:(half + 1) * QC],
                        start=(k == 0),
                        stop=(k == NK - 1),
                    )
                vdst = v_sb[st].rearrange("p (h c) -> p h c", h=H)[
                    :, half * 8:(half + 1) * 8, 0:DH
                ]
                vsrc = ps.rearrange("p (h c) -> p h c", h=8)
                nc.scalar.mul(vdst, vsrc, em[st])

        def scores_tiles(h):
            return [
                esp.tile([P, S], bf16, name=f"e{h}_{k}", tag="es") for k in range(NK)
            ]

        def emit_scores_half(h, esb, qt_t, kt_t, k0, k1):
            hs = (h % 2) * DH
            for k in range(k0, k1):
                ps = psA.tile([P, S], f32, name=f"s{h}_{k}", tag="psA")
                for qn in range(NQ):
                    nc.tensor.matmul(
                        ps[:, qn * QC:(qn + 1) * QC],
                        kt_t[hs:hs + DH, k * P:(k + 1) * P],
                        qt_t[hs:hs + DH, qn * QC:(qn + 1) * QC],
                        start=True,
                        stop=True,
                    )
                nc.scalar.activation(esb[k], ps, Exp, scale=SCALE)

        def emit_pv_half(h, esb, qn):
            # ctx'^T [65, S_q] = V'.T @ expS^T; SBUF bf16 copy (DVE)
            psc = psC.tile([VW, QC], f32, name=f"c{h}_{qn}", tag="psC")
            for k in range(NK):
                nc.tensor.matmul(
                    psc,
                    v_sb[k][:, h * VP:h * VP + VW],
                    esb[k][:, qn * QC:(qn + 1) * QC],
                    start=(k == 0),
                    stop=(k == NK - 1),
                )
            ct = ctp.tile([VW, QC], bf16, name=f"ct{h}_{qn}", tag="ct")
            nc.vector.tensor_copy(ct, psc)
            return ct

        def emit_ctxt_qn(h, ct, qn, split_muls=False):
            # 4 bf16 PE transposes per PSUM tile back to [S_q, 65];
            # one strided reciprocal per 4 denominators; normalize on DVE
            VW2 = VW + 1  # 66: keeps each chunk's PSUM byte offset 4B-aligned
            pst = psB.tile([P, 4 * VW2], bf16, name=f"pt{h}_{qn}", tag="psB")
            for qs in range(QC // P):
                nc.tensor.transpose(
                    pst[:, qs * VW2:qs * VW2 + VW],
                    ct[:, qs * P:(qs + 1) * P],
                    id_bf[0:VW, 0:VW],
                )
            rec = smallp.tile([P, 4], f32, name=f"r{h}_{qn}", tag="rec")
            pst4 = pst.rearrange("p (c w) -> p c w", w=VW2)
            nc.vector.reciprocal(rec, pst4[:, 0:4, DH])
            for qs in range(QC // P):
                qi = qn * (QC // P) + qs
                if split_muls and qs % 2:
                    nc.scalar.mul(
                        ctx_sb[qi][:, h * DH:(h + 1) * DH],
                        pst[:, qs * VW2:qs * VW2 + DH],
                        rec[:, qs:qs + 1],
                    )
                else:
                    nc.vector.tensor_scalar_mul(
                        ctx_sb[qi][:, h * DH:(h + 1) * DH],
                        pst[:, qs * VW2:qs * VW2 + DH],
                        rec[:, qs:qs + 1],
                    )

        def emit_out_cols(c0, c1):
            # columns [c0, c1) final for every row: one 3D DMA for all tiles
            nc.scalar.dma_start(
                out=o_d[:, c0:c1].rearrange("(st p) c -> p st c", p=P),
                in_=ctx_all[:, :, c0:c1],
            )

        # PE order: X^T, QK0, sc0, sc1, V, PV0, then per pair p>=1:
        #   QK(p), sc(2p)/PV(2p-1) halves interleaved, ctxT(2p-2),
        #   sc(2p+1) halves with ctxT(2p-1) between, PV(2p)
        # QK0 interleaved with the last four X^T tiles: the kt n=0 half
        # only needs X^T columns 0:512 (tiles 0-3)
        qt0 = qktp.tile([P, S], bf16, name="qt0", tag="qt")
        kt0 = qktp.tile([P, S], bf16, name="kt0", tag="kt")
        ps_k = psA.tile([P, S], f32, name="pq0_1", tag="psA")
        ps_q = psA.tile([P, S], f32, name="pq0_0", tag="psA")
        for k in range(NK):
            nc.tensor.matmul(
                ps_k[:, 0:QC], wbf0[1][:, k, :], xt[:, k, 0:QC],
                start=(k == 0), stop=(k == NK - 1),
            )
        emit_xt(4)
        emit_xt(5)
        for k in range(NK):
            nc.tensor.matmul(
                ps_q[:, 0:QC], wbf0[0][:, k, :], xt[:, k, 0:QC],
                start=(k == 0), stop=(k == NK - 1),
            )
        emit_xt(6)
        emit_xt(7)
        for k in range(NK):
            nc.tensor.matmul(
                ps_k[:, QC:S], wbf0[1][:, k, :], xt[:, k, QC:S],
                start=(k == 0), stop=(k == NK - 1),
            )
        nc.vector.tensor_copy(kt0, ps_k)
        for k in range(NK):
            nc.tensor.matmul(
                ps_q[:, QC:S], wbf0[0][:, k, :], xt[:, k, QC:S],
                start=(k == 0), stop=(k == NK - 1),
            )
        nc.vector.tensor_copy(qt0, ps_q)

        # pair-1 W casts first (data already landed), then W_v casts;
        # pair-1 QK matmul chunks interleave with sc0 so ACT's exp drain is
        # covered by non-PSUM-A work
        wbf1 = []
        for t, wf in enumerate(wfs1):
            wb = wqkp.tile([P, NK, P], bf16, name=f"wb1_{t}", tag="wb")
            nc.vector.tensor_copy(wb, wf)
            wbf1.append(wb)
        for q in range(4):
            nc.vector.tensor_copy(wvb[:, 2 * q:2 * q + 2, :], wvfs[q])
        qt1, kt1, chunks1 = qk_chunks(1, wbf1)
        es0, es1 = scores_tiles(0), scores_tiles(1)
        for k in range(NK):
            emit_scores_half(0, es0, qt0, kt0, k, k + 1)
            chunks1[k]()
        wbf_store = {2: qk_load(2)}
        for k in range(NK):
            emit_scores_half(1, es1, qt0, kt0, k, k + 1)
            emit_v_st(k)
        es_prev = {1: es1}
        cts = {0: [emit_pv_half(0, es0, 0), emit_pv_half(0, es0, 1)]}
        qt_cur, kt_cur = qt1, kt1
        for hp in range(1, NHP):
            h_a, h_b = 2 * hp, 2 * hp + 1
            h_pv = 2 * hp - 1
            qt_t, kt_t = qt_cur, kt_cur
            if hp + 1 < NHP:
                qtn, ktn, chunks = qk_chunks(hp + 1, wbf_store.pop(hp + 1))
            else:
                qtn = ktn = None
                chunks = [None] * 8
            if hp + 2 < NHP:
                wbf_store[hp + 2] = qk_load(hp + 2)
            esb_pv = es_prev.pop(h_pv)
            ct_prev = cts.pop(2 * hp - 2)
            es_a = scores_tiles(h_a)

            def unit(h, esb, k, filler):
                emit_scores_half(h, esb, qt_t, kt_t, k, k + 1)
                if filler is not None:
                    filler()

            if hp < NHP - 1:
                unit(h_a, es_a, 0, chunks[0])
                unit(h_a, es_a, 1, chunks[1])
                unit(h_a, es_a, 2, lambda: cts.__setitem__(
                    "t0", emit_pv_half(h_pv, esb_pv, 0)))
                unit(h_a, es_a, 3, chunks[2])
                unit(h_a, es_a, 4, chunks[3])
                unit(h_a, es_a, 5, lambda: cts.__setitem__(
                    "t1", emit_pv_half(h_pv, esb_pv, 1)))
                unit(h_a, es_a, 6, lambda: (
                    emit_ctxt_qn(2 * hp - 2, ct_prev[0], 0),
                    emit_ctxt_qn(2 * hp - 2, ct_prev[1], 1)))
                unit(h_a, es_a, 7, chunks[4])
                es_b = scores_tiles(h_b)
                unit(h_b, es_b, 0, chunks[5])
                unit(h_b, es_b, 1, lambda: emit_ctxt_qn(h_pv, cts["t0"], 0))
                unit(h_b, es_b, 2, chunks[6])
                unit(h_b, es_b, 3, chunks[7])
                unit(h_b, es_b, 4, lambda: emit_ctxt_qn(h_pv, cts["t1"], 1))
                unit(h_b, es_b, 5, lambda: cts.__setitem__(
                    "a0", emit_pv_half(h_a, es_a, 0)))
                unit(h_b, es_b, 6, None)
                unit(h_b, es_b, 7, lambda: cts.__setitem__(
                    "a1", emit_pv_half(h_a, es_a, 1)))
            else:
                # last pair: no QK chunks; stagger PV(14) qn0 k-steps behind
                # sc14's exps and pull ctxT(13) earlier
                psc14 = {}

                def pv14_steps(k0, k1, es_a=es_a):
                    def f():
                        if "ps" not in psc14:
                            psc14["ps"] = psC.tile(
                                [VW, QC], f32, name="c14_0", tag="psC"
                            )
                        for k in range(k0, k1):
                            nc.tensor.matmul(
                                psc14["ps"],
                                v_sb[k][:, 14 * VP:14 * VP + VW],
                                es_a[k][:, 0:QC],
                                start=(k == 0),
                                stop=False,
                            )
                    return f

                unit(h_a, es_a, 0, None)
                unit(h_a, es_a, 1, None)
                unit(h_a, es_a, 2, lambda: cts.__setitem__(
                    "t0", emit_pv_half(h_pv, esb_pv, 0)))
                unit(h_a, es_a, 3, pv14_steps(0, 1))
                unit(h_a, es_a, 4, lambda: (
                    emit_ctxt_qn(2 * hp - 2, ct_prev[0], 0),
                    emit_ctxt_qn(2 * hp - 2, ct_prev[1], 1)))
                unit(h_a, es_a, 5, lambda: cts.__setitem__(
                    "t1", emit_pv_half(h_pv, esb_pv, 1)))
                unit(h_a, es_a, 6, pv14_steps(1, 3))
                unit(h_a, es_a, 7, lambda: emit_ctxt_qn(h_pv, cts["t0"], 0))
                es_b = scores_tiles(h_b)

                def pv14_finish():
                    for k in range(5, NK):
                        nc.tensor.matmul(
                            psc14["ps"],
                            v_sb[k][:, 14 * VP:14 * VP + VW],
                            es_a[k][:, 0:QC],
                            start=False,
                            stop=(k == NK - 1),
                        )
                    ct = ctp.tile([VW, QC], bf16, name="ct14_0t", tag="ct")
                    nc.vector.tensor_copy(ct, psc14["ps"])
                    cts["a0"] = ct
                    cts["a1"] = emit_pv_half(h_a, es_a, 1)

                unit(h_b, es_b, 0, pv14_steps(3, 5))
                unit(h_b, es_b, 1, lambda: emit_ctxt_qn(h_pv, cts["t1"], 1))
                unit(h_b, es_b, 2, pv14_finish)
                # last pair: no QK chunks; stagger PV(15) qn0 k-steps into
                # the score block as each exp lands, shrinking the tail
                psc15 = {}

                def pv15_steps(k0, k1, es_b=es_b):
                    def f():
                        if "ps" not in psc15:
                            psc15["ps"] = psC.tile(
                                [VW, QC], f32, name="c15_0", tag="psC"
                            )
                        for k in range(k0, k1):
                            nc.tensor.matmul(
                                psc15["ps"],
                                v_sb[k][:, 15 * VP:15 * VP + VW],
                                es_b[k][:, 0:QC],
                                start=(k == 0),
                                stop=False,
                            )
                    return f

                unit(h_b, es_b, 3, pv15_steps(0, 1))
                unit(h_b, es_b, 4, pv15_steps(1, 2))
                unit(h_b, es_b, 5, pv15_steps(2, 4))
                unit(h_b, es_b, 6, pv15_steps(4, 5))
                unit(h_b, es_b, 7, pv15_steps(5, 6))
                cts["psc15"] = psc15
            cts[h_a] = [cts.pop("a0"), cts.pop("a1")]
            cts.pop("t0"), cts.pop("t1")
            es_prev[h_b] = es_b
            qt_cur, kt_cur = qtn, ktn
            if hp == 3:
                emit_out_cols(0, 4 * DH)
            elif hp == 5:
                emit_out_cols(4 * DH, 8 * DH)
            elif hp == 7:
                emit_out_cols(8 * DH, 12 * DH)
                emit_out_cols(12 * DH, 14 * DH)
        es15 = es_prev.pop(15)
        ps15 = cts.pop("psc15")["ps"]
        for k in (6, 7):
            nc.tensor.matmul(
                ps15,
                v_sb[k][:, 15 * VP:15 * VP + VW],
                es15[k][:, 0:QC],
                start=False,
                stop=(k == 7),
            )
        ct15_0 = ctp.tile([VW, QC], bf16, name="ct15_0t", tag="ct")
        nc.vector.tensor_copy(ct15_0, ps15)
        ct15 = [ct15_0, emit_pv_half(15, es15, 1)]
        ct14 = cts.pop(14)
        emit_ctxt_qn(14, ct14[0], 0, split_muls=True)
        emit_ctxt_qn(14, ct14[1], 1, split_muls=True)
        emit_out_cols(14 * DH, 15 * DH)
        emit_ctxt_qn(15, ct15[0], 0, split_muls=True)
        emit_ctxt_qn(15, ct15[1], 1, split_muls=True)
        emit_out_cols(15 * DH, 16 * DH)

    nc.finalize()
    return nc


def _get_nc():
    if "nc" not in _NC_CACHE:
        _NC_CACHE["nc"] = _build_nc()
    return _NC_CACHE["nc"]


def _run(hidden_states, attention_mask, qkv_weight, trace=False, **trace_kw):
    from concourse.bass_utils import run_bass_kernel_spmd

    nc = _get_nc()
    hidden = np.ascontiguousarray(np.asarray(hidden_states, dtype=np.float32))
    mask = np.ascontiguousarray(
        np.asarray(attention_mask, dtype=np.float32).reshape(B, S)
    )
    w = np.ascontiguousarray(np.asarray(qkv_weight, dtype=np.float32))
    in_maps = [
        {"x": hidden[b], "w": w, "m": mask[b]} for b in range(B)
    ]
    res = run_bass_kernel_spmd(nc, in_maps, list(range(B)), trace=trace, **trace_kw)
    out = np.stack([np.asarray(res.results[b]["o"]) for b in range(B)], axis=0)
    return out.astype(np.float32), res


def kernel(hidden_states, attention_mask, qkv_weight):
    out, _ = _run(hidden_states, attention_mask, qkv_weight, trace=False)
    return out


if __name__ == "__main__":
    _build_nc()
    print("build ok")
